# revision 62
# baseline (speedup 1.0000x reference)
"""Trainium2 Bass kernel for RoPE multi-head attention (B=2, T=2048, D=1024, H=16).

Sharding: tensor-parallel over heads. Each of the 8 cores owns 2 heads
(128 of the 1024 qkv dims):
  - QKV projections: every core holds X^T (replicated) and its 128-column
    slice of Wq/Wk/Wv; computes Q^T, K^T (rotated via RoPE) and V for its
    heads only.
  - Attention: scores computed transposed (S^T = K @ Q^T, [keys, toks]) so
    that exp(S^T) can feed the P@V matmul directly as the moving operand.
    Causality: only lower-triangular key/tok blocks are computed; the
    diagonal 128x128 block is masked with a 0/1 triangle after exp.
    The softmax denominator is produced by the same PV matmul via a ones
    column appended to V (stationary operand is [keys, 65]).
  - Output projection: an AllToAll redistributes attention outputs from
    head-sharded to token-sharded; each core then computes its 512-token
    slice of the final output against the full Wo.
Host side only reshapes / transposes / casts / shards; all arithmetic
(including the RoPE sin/cos tables, computed from token_positions on the
ScalarEngine) happens on device.
"""

import sys
from contextlib import ExitStack

for _p in ("/opt/trn_rl_repo",):
    if _p not in sys.path:
        sys.path.append(_p)

import numpy as np
import ml_dtypes

import concourse.bacc as bacc
import concourse.bass as bass
import concourse.mybir as mybir
import concourse.tile as tile
from concourse.bass_utils import run_bass_kernel_spmd

# Problem constants (hardcoded per harness contract).
B, T, D, H, DH = 2, 2048, 1024, 16, 64
NCORES = 8
HPC = H // NCORES          # heads per core = 2
TOK = B * T                # 4096 flattened tokens
THETA = 10000.0
SCALE = 1.0 / 8.0          # 1/sqrt(DH)
KS = D // 128              # 8 contraction slices
NKT = T // 128             # 16 key tiles per batch
NCH = T // 512             # 4 tok chunks (512) per batch
TPC = TOK // NCORES        # 512 tokens per core for the output projection

BF16 = mybir.dt.bfloat16
F32 = mybir.dt.float32


def build_nc(skip_collective=False, dummy=False):
    nc = bacc.Bacc(
        "TRN2",
        target_bir_lowering=False,
        debug=False,
        num_devices=NCORES,
    )

    # ---- kernel I/O ----
    xt_d = nc.dram_tensor("xt", [D, TOK], BF16, kind="ExternalInput")
    wqt_d = nc.dram_tensor("wqt", [D, 128], BF16, kind="ExternalInput")
    wkt_d = nc.dram_tensor("wkt", [D, 128], BF16, kind="ExternalInput")
    wvt_d = nc.dram_tensor("wvt", [D, 128], BF16, kind="ExternalInput")
    wot_d = nc.dram_tensor("wot", [D, D], BF16, kind="ExternalInput")
    posf_d = nc.dram_tensor("posf", [1, T], F32, kind="ExternalInput")
    out_d = nc.dram_tensor("out", [D, TPC], F32, kind="ExternalOutput")

    # ---- compile-time constants ----
    inv_freq = (1.0 / THETA ** (np.arange(DH // 2, dtype=np.float64) / (DH // 2)))
    invf4 = np.tile(inv_freq.astype(np.float32), 4)[None, :]  # [1, 128]
    invf_d = nc.inline_tensor(invf4, "invf")
    # tri[j, i] = 1 iff key j <= tok i (keeps lower-triangular attention).
    tri = np.triu(np.ones((128, 128), np.float32)).astype(ml_dtypes.bfloat16)
    tri_d = nc.inline_tensor(tri, "tri")

    if dummy:
        # identical I/O signature, near-empty body: used to measure the
        # fixed dispatch/NEFF-launch overhead so it can be subtracted
        with tile.TileContext(nc) as tc, ExitStack() as ctx:
            sp = ctx.enter_context(tc.tile_pool(name="sp", bufs=1))
            z = sp.tile([128, 16], F32, tag="z")
            nc.vector.memset(z, 0.0)
            nc.sync.dma_start(out=out_d[0:128, 0:16], in_=z)
        nc.compile()
        return nc

    with tile.TileContext(nc) as tc, ExitStack() as ctx:
        singles = ctx.enter_context(tc.tile_pool(name="singles", bufs=1))
        tmp = ctx.enter_context(tc.tile_pool(name="tmp", bufs=2))
        ppool = ctx.enter_context(tc.tile_pool(name="ppool", bufs=6))
        apool = ctx.enter_context(tc.tile_pool(name="apool", bufs=4))
        ps_main = ctx.enter_context(tc.tile_pool(name="ps_main", bufs=2, space="PSUM"))
        ps_pv = ctx.enter_context(tc.tile_pool(name="ps_pv", bufs=4, space="PSUM"))
        dpool = ctx.enter_context(tc.tile_pool(name="dram", bufs=1, space="DRAM"))

        # ---- persistent SBUF tensors ----
        # small inputs first so they don't queue behind the 8MB X^T load
        posf = singles.tile([1, T], F32, tag="posf")
        nc.sync.dma_start(out=posf, in_=posf_d.ap())
        invf = singles.tile([1, 128], F32, tag="invf")
        nc.sync.dma_start(out=invf, in_=invf_d.ap())
        tri_sb = singles.tile([128, 128], BF16, tag="tri")
        nc.sync.dma_start(out=tri_sb, in_=tri_d.ap())
        wq = singles.tile([128, KS, 128], BF16, tag="wq")
        nc.sync.dma_start(out=wq, in_=wqt_d.ap().rearrange("(k p) m -> p k m", p=128))
        wk = singles.tile([128, KS, 128], BF16, tag="wk")
        nc.sync.dma_start(out=wk, in_=wkt_d.ap().rearrange("(k p) m -> p k m", p=128))
        wv = singles.tile([128, KS, 128], BF16, tag="wv")
        nc.sync.dma_start(out=wv, in_=wvt_d.ap().rearrange("(k p) m -> p k m", p=128))
        xt = singles.tile([128, KS, TOK], BF16, tag="xt")
        xt_r = xt_d.ap().rearrange("(k p) t -> p k t", p=128)
        for k in range(KS):
            nc.sync.dma_start(out=xt[:, k, :], in_=xt_r[:, k, :])
        wo = singles.tile([128, KS, D], BF16, tag="wo")
        nc.sync.dma_start(out=wo, in_=wot_d.ap().rearrange("(k p) f -> p k f", p=128))

        qt = singles.tile([128, TOK], BF16, tag="qt")     # rotated Q^T
        kt = singles.tile([128, TOK], BF16, tag="kt")     # rotated K^T
        # V per (batch, keytile): [keys=128, 65*HPC]; col 64/129 = ones.
        vsb = singles.tile([128, B, NKT, 65 * HPC], BF16, tag="vsb")
        nc.vector.memset(vsb[:, :, :, 64:65], 1.0)
        nc.vector.memset(vsb[:, :, :, 129:130], 1.0)
        # ones row at partition 64 (same partition the PV rowsum lands on),
        # used to broadcast 1/rowsum across the 64 head dims via a K=1 matmul
        ones65 = singles.tile([65, 64], F32, tag="ones65")
        nc.vector.memset(ones65[64:65, :], 1.0)

        # cos table replicated over 4 row-blocks; sin table with sign baked
        # per half: rows [0:32] -sin, [32:64] +sin, then repeated.
        cs4 = singles.tile([128, T], F32, tag="cs4")
        sns4 = singles.tile([128, T], F32, tag="sns4")


        # ---- RoPE tables: ang = pos * inv_freq, sin/cos on ScalarE ----
        # ScalarE Sin needs args in [-pi, pi]. Range-reduce with the fp32
        # magic-number round: rn(x) = (x + 1.5*2^23) - 1.5*2^23, so
        # red = ang - 2pi*rn(ang/2pi) lands in [-pi, pi] (clamped for safety).
        tau = float(2 * np.pi)
        magic = float(1.5 * 2 ** 23)

        def reduced_sin(dst, src_ap):
            qm = tmp.tile([128, 512], F32, tag="ra", name="qm", bufs=1)
            nc.vector.tensor_scalar(
                out=qm, in0=src_ap, scalar1=1.0 / tau, scalar2=magic,
                op0=mybir.AluOpType.mult, op1=mybir.AluOpType.add,
            )
            qr = tmp.tile([128, 512], F32, tag="rb", name="qr", bufs=1)
            nc.vector.tensor_scalar(
                out=qr, in0=qm, scalar1=magic, scalar2=None,
                op0=mybir.AluOpType.subtract,
            )
            red = tmp.tile([128, 512], F32, tag="rc", name="red", bufs=1)
            nc.vector.scalar_tensor_tensor(
                out=red, in0=qr, scalar=-tau, in1=src_ap,
                op0=mybir.AluOpType.mult, op1=mybir.AluOpType.add,
            )
            redc = tmp.tile([128, 512], F32, tag="rd", name="redc", bufs=1)
            nc.vector.tensor_scalar(
                out=redc, in0=red, scalar1=float(np.pi), scalar2=float(-np.pi),
                op0=mybir.AluOpType.min, op1=mybir.AluOpType.max,
            )
            nc.scalar.activation(
                out=dst, in_=redc, func=mybir.ActivationFunctionType.Sin
            )

        for q4 in range(T // 512):
            ps_a = ps_main.tile([128, 512], F32, tag="big")
            nc.tensor.matmul(
                ps_a, invf, posf[:, q4 * 512:(q4 + 1) * 512], start=True, stop=True
            )
            sl = slice(q4 * 512, (q4 + 1) * 512)
            reduced_sin(sns4[:, sl], ps_a)
            angc = tmp.tile([128, 512], F32, tag="re", name="angc", bufs=1)
            nc.vector.tensor_scalar(
                out=angc, in0=ps_a, scalar1=float(np.pi / 2), scalar2=None,
                op0=mybir.AluOpType.add,
            )
            reduced_sin(cs4[:, sl], angc)
        # bake the rotate-half signs: rows [0:32] and [64:96] get -sin
        nc.vector.tensor_scalar(
            out=sns4[0:32, :], in0=sns4[0:32, :], scalar1=-1.0, scalar2=None,
            op0=mybir.AluOpType.mult,
        )
        nc.vector.tensor_scalar(
            out=sns4[64:96, :], in0=sns4[64:96, :], scalar1=-1.0, scalar2=None,
            op0=mybir.AluOpType.mult,
        )

        # ---- QKV projections (+ RoPE rotation for Q, K) ----
        def project_rot(w_sb, dst, ch):
            # one 1024-token chunk: psum = (W_c^T)^T-style accumulation
            ps = ps_main.tile([128, 1024], F32, tag="big")
            for k in range(KS):
                for hh in range(2):
                    nc.tensor.matmul(
                        ps[:, hh * 512:(hh + 1) * 512],
                        w_sb[:, k, :],
                        xt[:, k, ch * 1024 + hh * 512: ch * 1024 + (hh + 1) * 512],
                        start=(k == 0),
                        stop=(k == KS - 1),
                    )
            # RoPE (rotate-half): rows h*64+[0:32] = x1 (even dims),
            # h*64+[32:64] = x2 (odd dims).  rot = x*cos + swap(x)*sns where
            # swap exchanges the two 32-row halves of each head (via DMA,
            # since DVE lanes cannot cross partitions).
            ts = (ch % (T // 1024)) * 1024  # position within batch
            t1 = tmp.tile([128, 1024], BF16, tag="t1")
            nc.vector.tensor_mul(t1, ps, cs4[:, ts:ts + 1024])
            xc = tmp.tile([128, 1024], BF16, tag="xc")
            nc.scalar.copy(out=xc, in_=ps)
            xs = tmp.tile([128, 1024], BF16, tag="xs")
            for h in range(HPC):
                o = h * 64
                nc.sync.dma_start(out=xs[o:o + 32, :], in_=xc[o + 32:o + 64, :])
                nc.sync.dma_start(out=xs[o + 32:o + 64, :], in_=xc[o:o + 32, :])
            t2 = tmp.tile([128, 1024], BF16, tag="t2")
            nc.gpsimd.tensor_mul(t2, xs, sns4[:, ts:ts + 1024])
            cols = slice(ch * 1024, (ch + 1) * 1024)
            nc.vector.tensor_add(dst[:, cols], t1, t2)

        for ch in range(TOK // 1024):
            project_rot(wq, qt, ch)
            project_rot(wk, kt, ch)

        # ---- attention (per batch, per 1024-token group) ----
        a2a_in = dpool.tile([NCORES, 128, TPC], BF16, tag="a2a_in")
        a2a_out = dpool.tile([NCORES, 128, TPC], BF16, tag="a2a_out")

        # V production is interleaved into the attention loop as filler PE
        # work: each produce_v is ~8 dense matmuls with no dependency on
        # exp, lengthening per-keytile PE bursts past the ~3.4us HAM
        # warm-up threshold so attention matmuls run at full clock.
        vjobs = [(b_, kt_) for b_ in range(B) for kt_ in range(NKT)]

        def produce_v(bv, ktv):
            ps_v = ps_main.tile([128, 1024], F32, tag="big", name="ps_v")
            t0 = bv * T + ktv * 128
            for k in range(KS):
                nc.tensor.matmul(
                    ps_v[:, 0:128],
                    xt[:, k, t0:t0 + 128],
                    wv[:, k, :],
                    start=(k == 0),
                    stop=(k == KS - 1),
                )
            for h in range(HPC):
                nc.vector.tensor_copy(
                    out=vsb[:, bv, ktv, 65 * h:65 * h + 64],
                    in_=ps_v[:, h * 64:(h + 1) * 64],
                )

        for b in range(B):
            for c2 in range(T // 1024):
                # ensure every V keytile this group consumes is produced
                while vjobs and (vjobs[0][0] < b or (
                        vjobs[0][0] == b and vjobs[0][1] < 8 * (c2 + 1))):
                    produce_v(*vjobs.pop(0))
                # live PV accumulators: [65, 512] per (head, chunk-half)
                pv = {}
                for h in range(HPC):
                    for cl in range(2):
                        pv[(h, cl)] = ps_pv.tile(
                            [65, 512], F32, tag="pv", name=f"pv{h}{cl}"
                        )
                # software-pipelined: emit PV matmuls of keytile k-1 after
                # the scores+exp of keytile k, so PE's in-order stream never
                # stalls on the exp/mask it is itself supposed to overlap
                pv_work = []

                def emit_pv(work):
                    h, ktile, p_sb = work
                    for cl in range(2):
                        c = 2 * c2 + cl  # global 512-chunk in batch
                        if ktile > 4 * c + 3:
                            continue
                        nc.tensor.matmul(
                            pv[(h, cl)],
                            vsb[:, b, ktile, 65 * h:65 * h + 65],
                            p_sb[:, cl * 512:(cl + 1) * 512],
                            start=(ktile == 0),
                            stop=(ktile == 4 * c + 3),
                        )

                for ktile in range(8 * (c2 + 1)):
                    if vjobs:  # filler: keep the PE dense between exp waits
                        produce_v(*vjobs.pop(0))
                    ts0 = max(0, ktile * 128 - c2 * 1024)  # first valid tok
                    for h in range(HPC):
                        o = h * 64
                        ps_s = ps_main.tile([128, 1024], F32, tag="big")
                        for hh in range(ts0 // 512, 2):
                            nc.tensor.matmul(
                                ps_s[:, hh * 512:(hh + 1) * 512],
                                kt[o:o + 64, b * T + ktile * 128: b * T + ktile * 128 + 128],
                                qt[o:o + 64, b * T + c2 * 1024 + hh * 512: b * T + c2 * 1024 + (hh + 1) * 512],
                                start=True,
                                stop=True,
                            )
                        p_sb = ppool.tile([128, 1024], BF16, tag="p")
                        nc.scalar.activation(
                            out=p_sb[:, ts0:1024], in_=ps_s[:, ts0:1024],
                            func=mybir.ActivationFunctionType.Exp, scale=SCALE,
                        )
                        # mask the diagonal block (keys > tok -> 0); only when
                        # this keytile's diagonal falls inside this tok window
                        if ktile * 128 >= c2 * 1024:
                            nc.vector.tensor_mul(
                                p_sb[:, ts0:ts0 + 128], p_sb[:, ts0:ts0 + 128], tri_sb
                            )
                        # zero the pre-diagonal gap inside the first chunk-half
                        g0 = (ts0 // 512) * 512
                        if ts0 > g0:
                            nc.gpsimd.memset(p_sb[:, g0:ts0], 0.0)
                        pv_work.append((h, ktile, p_sb))
                        if len(pv_work) > 2:
                            emit_pv(pv_work.pop(0))
                for w in pv_work:
                    emit_pv(w)
                # normalize + ship to a2a buffer
                for h in range(HPC):
                    for cl in range(2):
                        c = 2 * c2 + cl
                        rcp = tmp.tile([65, 512], F32, tag="rcp")
                        nc.vector.reciprocal(rcp[64:65, :], pv[(h, cl)][64:65, :])
                        bc_ps = ps_main.tile([64, 512], F32, tag="big", name="bc_ps")
                        nc.tensor.matmul(
                            bc_ps, ones65[64:65, :], rcp[64:65, :],
                            start=True, stop=True,
                        )
                        bc = tmp.tile([64, 512], F32, tag="bc")
                        nc.vector.tensor_copy(out=bc, in_=bc_ps)
                        att = apool.tile([64, 512], BF16, tag="att")
                        nc.vector.tensor_mul(att, pv[(h, cl)][0:64, :], bc)
                        nc.sync.dma_start(
                            out=a2a_in[b * NCH + c, h * 64:(h + 1) * 64, :], in_=att
                        )

        # ---- AllToAll: head-sharded -> token-sharded ----
        if skip_collective:
            # timeline-model variant: stand-in DRAM copy instead of the
            # collective (TimelineSim cannot model collectives)
            nc.sync.dma_start(out=a2a_out[:], in_=a2a_in[:])
        else:
            nc.gpsimd.collective_compute(
                "AllToAll",
                mybir.AluOpType.bypass,
                replica_groups=[list(range(NCORES))],
                ins=[a2a_in[:]],
                outs=[a2a_out[:]],
            )

        # ---- output projection for this core's 512 tokens ----
        ao = singles.tile([128, KS, TPC], BF16, tag="ao")
        for s in range(KS):
            nc.sync.dma_start(out=ao[:, s, :], in_=a2a_out[s, :, :])
        for m in range(KS):
            ps_o = ps_main.tile([128, 1024], F32, tag="big")
            for s in range(KS):
                nc.tensor.matmul(
                    ps_o[:, 0:TPC],
                    wo[:, s, m * 128:(m + 1) * 128],
                    ao[:, s, :],
                    start=(s == 0),
                    stop=(s == KS - 1),
                )
            osb = tmp.tile([128, TPC], F32, tag="osb")
            nc.vector.tensor_copy(out=osb, in_=ps_o[:, 0:TPC])
            nc.sync.dma_start(out=out_d[m * 128:(m + 1) * 128, :], in_=osb)

    nc.compile()
    return nc


_NC = None


def _get_nc():
    global _NC
    if _NC is None:
        _NC = build_nc()
    return _NC


def _rope_perm():
    # per-head deinterleave: evens first then odds, applied to a 128-row slice
    p = []
    for h in range(HPC):
        p.extend(h * DH + np.arange(0, DH, 2))
        p.extend(h * DH + np.arange(1, DH, 2))
    return np.asarray(p)


def make_in_maps(inputs):
    x = np.asarray(inputs["in_features"], np.float32)
    Wq = np.asarray(inputs["Wq"], np.float32)
    Wk = np.asarray(inputs["Wk"], np.float32)
    Wv = np.asarray(inputs["Wv"], np.float32)
    Wo = np.asarray(inputs["Wo"], np.float32)
    pos = np.asarray(inputs["token_positions"]).astype(np.float32)[None, :]

    bf = ml_dtypes.bfloat16
    XT = np.ascontiguousarray(x.reshape(TOK, D).T).astype(bf)
    WoT = np.ascontiguousarray(Wo.T).astype(bf)
    perm = _rope_perm()

    in_maps = []
    for c in range(NCORES):
        rows = slice(c * 128, (c + 1) * 128)
        wq_c = Wq[rows][perm]
        wk_c = Wk[rows][perm]
        wv_c = Wv[rows]
        in_maps.append({
            "xt": XT,
            "wqt": np.ascontiguousarray(wq_c.T).astype(bf),
            "wkt": np.ascontiguousarray(wk_c.T).astype(bf),
            "wvt": np.ascontiguousarray(wv_c.T).astype(bf),
            "wot": WoT,
            "posf": pos,
        })
    return in_maps


def assemble_out(results):
    full = np.empty((TOK, D), np.float32)
    for c in range(NCORES):
        chunk = np.asarray(results[c]["out"])  # [D, 512] = out^T slice
        t0 = (c // 4) * T + (c % 4) * TPC
        full[t0:t0 + TPC] = chunk.T
    return full.reshape(B, T, D)


def run(inputs, **kwargs):
    nc = _get_nc()
    res = run_bass_kernel_spmd(
        nc, make_in_maps(inputs), core_ids=list(range(NCORES)), **kwargs
    )
    return assemble_out(res.results), res


def kernel(**inputs) -> np.ndarray:
    out, _ = run(inputs)
    return out


# ---------------------------------------------------------------------------
# Benchmark path: cached jitted executable so repeat executions can be timed
# without retracing/recompiling. Mirrors bass2jax.run_bass_via_pjrt.
# ---------------------------------------------------------------------------
_EXEC = {}


def _build_exec(kind="main"):
    if kind in _EXEC:
        return _EXEC[kind]
    import jax
    from jax.experimental.shard_map import shard_map
    from jax.sharding import Mesh, PartitionSpec

    import concourse.mybir as mybir
    from concourse import bass2jax

    nc = _get_nc() if kind == "main" else build_nc(dummy=True)
    bass2jax.install_neuronx_cc_hook()

    partition_name = nc.partition_id_tensor.name if nc.partition_id_tensor else None
    in_names, out_names, out_avals, zero_outs = [], [], [], []
    for alloc in nc.m.functions[0].allocations:
        if not isinstance(alloc, mybir.MemoryLocationSet):
            continue
        name = alloc.memorylocations[0].name
        if alloc.kind == "ExternalInput":
            if name != partition_name:
                in_names.append(name)
        elif alloc.kind == "ExternalOutput":
            out_names.append(name)
            shape = tuple(alloc.tensor_shape)
            dtype = mybir.dt.np(alloc.dtype)
            out_avals.append(jax.core.ShapedArray(shape, dtype))
            zero_outs.append(np.zeros(shape, dtype))
    n_params = len(in_names)
    all_names = list(in_names) + list(out_names)
    if partition_name is not None:
        all_names.append(partition_name)

    def _body(*args):
        outs = bass2jax._bass_exec_p.bind(
            *(list(args) + ([bass2jax.partition_id_tensor()]
                            if partition_name is not None else [])),
            out_avals=tuple(out_avals),
            in_names=tuple(all_names),
            out_names=tuple(out_names),
            lowering_input_output_aliases=(),
            sim_require_finite=True,
            sim_require_nnan=True,
            nc=nc,
        )
        return tuple(outs)

    devices = jax.devices()[:NCORES]
    mesh = Mesh(np.asarray(devices), ("core",))
    nspec = n_params + len(out_names)
    sharded = jax.jit(
        shard_map(
            _body,
            mesh=mesh,
            in_specs=(PartitionSpec("core"),) * nspec,
            out_specs=(PartitionSpec("core"),) * len(out_names),
            check_rep=False,
        ),
        keep_unused=True,
    )
    _EXEC[kind] = (sharded, in_names, out_names, zero_outs, mesh)
    return _EXEC[kind]


def _stage_args(inputs, exec_tuple):
    import jax
    from jax.sharding import NamedSharding, PartitionSpec

    sharded, in_names, out_names, zero_outs, mesh = exec_tuple
    in_maps = make_in_maps(inputs)
    sh = NamedSharding(mesh, PartitionSpec("core"))
    args = []
    for name in in_names:
        cat = np.concatenate([in_maps[c][name] for c in range(NCORES)], axis=0)
        args.append(jax.device_put(cat, sh))
    for z in zero_outs:
        cat = np.concatenate([z] * NCORES, axis=0)
        args.append(jax.device_put(cat, sh))
    return args


def _timed(fn, args, iters):
    import time

    import jax

    jax.block_until_ready(fn(*args))
    best = float("inf")
    for _ in range(iters):
        t0 = time.perf_counter()
        outs = fn(*args)
        jax.block_until_ready(outs)
        best = min(best, time.perf_counter() - t0)
    return outs, best


def run_bench(inputs, iters=10):
    """Returns (output, est_exec_seconds, t_full, t_dummy): times the real
    kernel and a near-empty NEFF with identical I/O; the difference cancels
    the axon dispatch + NEFF-launch overhead."""
    e1 = _build_exec("main")
    args = _stage_args(inputs, e1)
    outs, t_full = _timed(e1[0], args, iters)
    ed = _build_exec("dummy")
    _timed(ed[0], args, iters)
    _, t_dummy = _timed(ed[0], args, iters)

    _, in_names, out_names, zero_outs, mesh = e1
    results = []
    for c in range(NCORES):
        m = {}
        for i, name in enumerate(out_names):
            arr = np.asarray(outs[i])
            per = arr.shape[0] // NCORES
            m[name] = arr[c * per:(c + 1) * per]
        results.append(m)
    return assemble_out(results), max(t_full - t_dummy, 0.0), t_full, t_dummy

